# revision 7
# baseline (speedup 1.0000x reference)
"""Causal single-head attention (B=4, S=4096, D=512, dk=64) on 8 Trainium2
NeuronCores via Bass/Tile.

Sharding: core c handles batch b = c//2, query parity p = c%2 — the four
512-row query chunks with global chunk index 2j+p, j=0..3.  Work per job j
is uniform across cores (E[j] = 8j+8 key-tiles of 128); causal boundary
differences between parities are handled by per-core 0/1 mask tensors
(data, not program), so a single SPMD program serves all 8 cores.

Per-core pipeline:
  qT = relu(Wq^T x1T + bq)   [64, 2048]   (PE, ACT relu+bias)
  kT = relu(Wk^T x2T + bk)   [64, 4096]
  vT = relu(Wv^T x2T + bv) -> PE-transposed into v_aug tiles [128, 65]
       (column 64 = 1.0 so the PV matmul also accumulates the softmax
        denominator)
  for each key tile t (sk-outer):            scT = kT_tile^T qT  (PSUM)
      attnT = exp(scT / 8)  (ACT, no max-subtraction: scores in [0, ~6])
      diagonal-band jobs multiply by a mask tile (DVE)
      outT[j] += v_aug_tile^T attnT          (PSUM accum over t)
  finalize: PE-transpose outT -> natural layout, divide by denominator
  column (DVE reciprocal + per-partition scalar multiply), DMA out.

Matmul inputs are float32r (full fp32 bits in DRAM/SBUF; the PE rounds on
read) so matmuls run at 1 cycle/row instead of fp32's 4.
"""
import os
import numpy as np

import bass_rust
import concourse.bass as bass
import concourse.tile as tile
from concourse import mybir
from concourse.bass_utils import run_bass_kernel_spmd
from concourse.masks import make_identity

# ---------------------------------------------------------------- constants
P = 128          # partitions / sk tile
D = 512          # model dim
DK = 64          # key dim
S = 4096         # sequence
B = 4            # batch
CH = 512         # sq chunk width (one job)
NJ = 4           # jobs per core
KD = D // P      # k-tiles in the D contraction
NSK = S // P     # sk tiles
SQ = NJ * CH     # q rows per core
N_CORES = 8

F32 = mybir.dt.float32
F32R = mybir.dt.float32r

_CFG = {
    "f32r_proj": os.environ.get("K_F32R_PROJ", "1") == "1",
    "f32r_sc": os.environ.get("K_F32R_SC", "1") == "1",
    "f32r_pv": os.environ.get("K_F32R_PV", "1") == "1",
    "trace": os.environ.get("K_TRACE", "0") == "1",
}


# ------------------------------------------------- walrus codegen workarounds
def _patch_tile_drain():
    """This neuronxcc rejects >1 sync wait on a CTRL (Drain) instruction;
    TileContext's tail drain carries one wait per live semaphore.  Split the
    waits onto dedicated SP nops, one wait each."""
    from concourse.tile import TileContext

    if getattr(TileContext, "_drain_patched", False):
        return

    def _patched(self, tick_clock, wait_clock):
        nc = self.nc
        probe = nc.sync.nop(nofuse=True, hint="tail_wait_probe")
        wait_clock.add_sem_waits(
            probe.ins, bass_rust.ScopedClock({None: tick_clock.global_clock})
        )
        si = probe.ins.sync_info
        waits = list(si.on_wait) if si is not None else []
        probe.ins.sync_info = bass_rust.SyncInfo(on_wait=waits[:1], on_update=[])
        for w in waits[1:]:
            carrier = nc.sync.nop(nofuse=True, hint="tail_wait")
            carrier.ins.sync_info = bass_rust.SyncInfo(on_wait=[w], on_update=[])
        nc.sync.drain()

        nc.all_engine_barrier()
        assert self.sems is not None
        popped = nc._tile_sem_poison_stack.pop()
        assert popped is self._sem_poison
        nc.clear_and_free_semaphores(list(self.sems.allocated().values()))
        nc.all_engine_barrier()

    TileContext._drain_and_barrier = _patched
    TileContext._drain_patched = True


def _split_sync_waits(nc, max_waits: int = 1):
    """walrus here rejects >1 sync wait on at least CTRL and S3_LW (weight
    load) instruction structs.  Hoist excess waits onto same-engine NOPs
    placed immediately before the instruction (engine streams execute block
    order, so the waits still gate the instruction)."""
    counter = [0]
    for fn in nc.m.functions:
        for bb in fn.blocks:
            changed = False
            new = []
            for inst in bb.instructions:
                si = inst.sync_info
                waits = list(si.on_wait) if si is not None else []
                if len(waits) > max_waits:
                    changed = True
                    for w in waits[:-max_waits]:
                        counter[0] += 1
                        nop = bass_rust.InstNoOp(
                            name=f"I-waitsplit-{counter[0]}", engine=inst.engine
                        )
                        nop.bass_nofuse = True
                        nop.sync_info = bass_rust.SyncInfo(
                            on_wait=[w], on_update=[]
                        )
                        new.append(nop)
                    inst.sync_info = bass_rust.SyncInfo(
                        on_wait=waits[-max_waits:], on_update=list(si.on_update)
                    )
                new.append(inst)
            if changed:
                bb.instructions = new


# ---------------------------------------------------------------- program
def _build_program(causal: bool):
    _patch_tile_drain()
    nc = bass.Bass()

    DT_X = F32R if _CFG["f32r_proj"] else F32
    DT_QK = F32R if _CFG["f32r_sc"] else F32
    DT_AT = F32R if _CFG["f32r_pv"] else F32

    x1t = nc.declare_dram_parameter("x1t", [D, SQ], DT_X, isOutput=False)
    x2t = nc.declare_dram_parameter("x2t", [D, S], DT_X, isOutput=False)
    wq = nc.declare_dram_parameter("wq", [D, DK], DT_X, isOutput=False)
    wk = nc.declare_dram_parameter("wk", [D, DK], DT_X, isOutput=False)
    wv = nc.declare_dram_parameter("wv", [D, DK], DT_X, isOutput=False)
    bq = nc.declare_dram_parameter("bq", [DK], F32, isOutput=False)
    bk = nc.declare_dram_parameter("bk", [DK], F32, isOutput=False)
    bv = nc.declare_dram_parameter("bv", [DK], F32, isOutput=False)
    masks = nc.declare_dram_parameter("masks", [8, P, CH], F32, isOutput=False)
    ones = nc.declare_dram_parameter("ones", [P, NSK], DT_AT, isOutput=False)
    out = nc.declare_dram_parameter("out", [SQ, DK], F32, isOutput=True)

    E = [8 * j + 8 for j in range(NJ)] if causal else [NSK] * NJ

    Relu = mybir.ActivationFunctionType.Relu
    Exp = mybir.ActivationFunctionType.Exp

    with tile.TileContext(nc) as tc:
        with (
            tc.tile_pool(name="const", bufs=1) as const,
            tc.tile_pool(name="xin", bufs=3) as xin,
            tc.tile_pool(name="resident", bufs=1) as res,
            tc.tile_pool(name="attn", bufs=4) as attn,
            tc.tile_pool(name="ostage", bufs=4) as ostage,
            tc.tile_pool(name="ps", bufs=2, space="PSUM") as ps,
            tc.tile_pool(name="outps", bufs=1, space="PSUM") as outps,
        ):
            # ---------------- constants
            wq_sb = const.tile([P, KD, DK], DT_X)
            wk_sb = const.tile([P, KD, DK], DT_X)
            wv_sb = const.tile([P, KD, DK], DT_X)
            nc.sync.dma_start(out=wq_sb, in_=wq.rearrange("(kd p) m -> p kd m", p=P))
            nc.sync.dma_start(out=wk_sb, in_=wk.rearrange("(kd p) m -> p kd m", p=P))
            nc.sync.dma_start(out=wv_sb, in_=wv.rearrange("(kd p) m -> p kd m", p=P))
            bq_sb = const.tile([DK, 1], F32)
            bk_sb = const.tile([DK, 1], F32)
            bv_sb = const.tile([DK, 1], F32)
            nc.sync.dma_start(out=bq_sb, in_=bq.rearrange("(p o) -> p o", o=1))
            nc.sync.dma_start(out=bk_sb, in_=bk.rearrange("(p o) -> p o", o=1))
            nc.sync.dma_start(out=bv_sb, in_=bv.rearrange("(p o) -> p o", o=1))
            if causal:
                masks_sb = const.tile([P, 8, CH], F32)
                nc.sync.dma_start(out=masks_sb, in_=masks.rearrange("m p s -> p m s"))
            ident = const.tile([P, P], F32)
            make_identity(nc, ident)

            # ---------------- qT projection: [64, 2048]
            qT_sb = res.tile([DK, SQ], DT_QK)
            x1v = x1t.rearrange("(kd p) s -> p kd s", p=P)
            for ch in range(SQ // CH):
                xt = xin.tile([P, KD, CH], DT_X, tag="x1c")
                nc.sync.dma_start(out=xt, in_=x1v[:, :, ch * CH:(ch + 1) * CH])
                pq = ps.tile([DK, CH], F32, tag="ps")
                for kd in range(KD):
                    nc.tensor.matmul(
                        pq, wq_sb[:, kd, :], xt[:, kd, :],
                        start=(kd == 0), stop=(kd == KD - 1),
                    )
                nc.scalar.activation(
                    out=qT_sb[:, ch * CH:(ch + 1) * CH], in_=pq, func=Relu,
                    bias=bq_sb, scale=1.0,
                )

            # ---------------- kT / v projections over full S
            kT_sb = res.tile([DK, S], DT_QK)
            v_sb = res.tile([P, NSK, DK + 1], DT_AT)
            nc.sync.dma_start(
                out=v_sb[:, :, DK:DK + 1],
                in_=ones.rearrange("p (n o) -> p n o", o=1),
            )
            x2v = x2t.rearrange("(kd p) s -> p kd s", p=P)
            for ch in range(S // CH):
                xt = xin.tile([P, KD, CH], DT_X, tag="x2c")
                nc.sync.dma_start(out=xt, in_=x2v[:, :, ch * CH:(ch + 1) * CH])
                pk = ps.tile([DK, CH], F32, tag="ps")
                for kd in range(KD):
                    nc.tensor.matmul(
                        pk, wk_sb[:, kd, :], xt[:, kd, :],
                        start=(kd == 0), stop=(kd == KD - 1),
                    )
                nc.scalar.activation(
                    out=kT_sb[:, ch * CH:(ch + 1) * CH], in_=pk, func=Relu,
                    bias=bk_sb, scale=1.0,
                )
                pv = ps.tile([DK, CH], F32, tag="ps")
                for kd in range(KD):
                    nc.tensor.matmul(
                        pv, wv_sb[:, kd, :], xt[:, kd, :],
                        start=(kd == 0), stop=(kd == KD - 1),
                    )
                vt = attn.tile([DK, CH], F32, tag="vT")
                nc.scalar.activation(out=vt, in_=pv, func=Relu, bias=bv_sb, scale=1.0)
                for blk in range(CH // P):
                    st = ch * (CH // P) + blk
                    pt = ps.tile([P, DK], F32, tag="ps")
                    nc.tensor.transpose(
                        pt, in_=vt[:, blk * P:(blk + 1) * P], identity=ident[:DK, :DK]
                    )
                    nc.vector.tensor_copy(v_sb[:, st, 0:DK], pt)

            # ---------------- attention, sk-outer
            outT = [
                outps.tile([DK + 1, CH], F32, name=f"outT{j}", tag=f"outT{j}")
                for j in range(NJ)
            ]
            for t in range(NSK):
                jobs = [j for j in range(NJ) if E[j] > t]
                kslc = kT_sb[:, t * P:(t + 1) * P]
                # pair jobs into [128, 1024] psum tiles to halve ACT overhead
                for i0 in range(0, len(jobs), 2):
                    pair = jobs[i0:i0 + 2]
                    w = len(pair) * CH
                    sc = ps.tile([P, 1024], F32, tag="ps")
                    at = attn.tile([P, 1024], DT_AT, tag="attnT")
                    for idx, j in enumerate(pair):
                        nc.tensor.matmul(
                            sc[:, idx * CH:(idx + 1) * CH],
                            kslc,
                            qT_sb[:, j * CH:(j + 1) * CH],
                            start=True,
                            stop=True,
                        )
                    nc.scalar.activation(
                        out=at[:, :w], in_=sc[:, :w], func=Exp, scale=0.125
                    )
                    for idx, j in enumerate(pair):
                        aslc = at[:, idx * CH:(idx + 1) * CH]
                        if causal and j == t // 8:
                            nc.vector.tensor_tensor(
                                aslc, aslc, masks_sb[:, t % 8, :],
                                mybir.AluOpType.mult,
                            )
                        nc.tensor.matmul(
                            outT[j],
                            v_sb[:, t, :],
                            aslc,
                            start=(t == 0),
                            stop=(t == E[j] - 1),
                            skip_group_check=True,
                        )
                # finalize completed jobs
                for j in range(NJ):
                    if E[j] - 1 != t:
                        continue
                    oT = ostage.tile([DK + 1, CH], F32, tag="oT")
                    nc.vector.tensor_copy(oT, outT[j])
                    for blk in range(CH // P):
                        po = ps.tile([P, DK + 1], F32, tag="ps")
                        nc.tensor.transpose(
                            po,
                            in_=oT[:, blk * P:(blk + 1) * P],
                            identity=ident[:DK + 1, :DK + 1],
                        )
                        rec = ostage.tile([P, 1], F32, tag="rec")
                        nc.vector.reciprocal(rec, po[:, DK:DK + 1])
                        ot = ostage.tile([P, DK], F32, tag="ot")
                        nc.vector.tensor_scalar_mul(ot, po[:, 0:DK], rec)
                        r0 = j * CH + blk * P
                        nc.sync.dma_start(out=out[r0:r0 + P, :], in_=ot)

    _split_sync_waits(nc)
    return nc


_PROGRAMS = {}


def _program(causal: bool):
    if causal not in _PROGRAMS:
        _PROGRAMS[causal] = _build_program(causal)
    return _PROGRAMS[causal]


def _host_masks(parity: int) -> np.ndarray:
    """masks[m] multiplies the exp'd [sk=128, sq=512] tile of the job whose
    diagonal band covers key tiles [E-8, E); m = position in that band."""
    sk = np.arange(P)[:, None]
    sq = np.arange(CH)[None, :]
    m = np.zeros((8, P, CH), np.float32)
    for i in range(8):
        if parity == 1:
            if i < 4:
                m[i] = 1.0
            else:
                r = i - 4
                m[i] = (sq >= r * P + sk).astype(np.float32)
        else:
            if i < 4:
                m[i] = (sq >= i * P + sk).astype(np.float32)
            else:
                m[i] = 0.0
    return m


def kernel(x1, x2, Wq, bq, Wk, bk, Wv, bv, apply_mask):
    x1 = np.asarray(x1, dtype=np.float32)
    x2 = np.asarray(x2, dtype=np.float32)
    Wq = np.ascontiguousarray(np.asarray(Wq, np.float32))
    Wk = np.ascontiguousarray(np.asarray(Wk, np.float32))
    Wv = np.ascontiguousarray(np.asarray(Wv, np.float32))
    bq = np.ascontiguousarray(np.asarray(bq, np.float32))
    bk = np.ascontiguousarray(np.asarray(bk, np.float32))
    bv = np.ascontiguousarray(np.asarray(bv, np.float32))
    causal = bool(int(np.asarray(apply_mask)))

    nc = _program(causal)

    in_maps = []
    for core in range(N_CORES):
        b, p = core // 2, core % 2
        xb = x1[b]                                   # [S, D]
        rows = np.concatenate(
            [xb[(2 * j + p) * CH:(2 * j + p + 1) * CH] for j in range(NJ)], axis=0
        )                                            # [2048, D]
        in_maps.append({
            "x1t": np.ascontiguousarray(rows.T),     # [D, 2048]
            "x2t": np.ascontiguousarray(x2[b].T),    # [D, 4096]
            "wq": Wq, "wk": Wk, "wv": Wv,
            "bq": bq, "bk": bk, "bv": bv,
            "masks": _host_masks(p),
            "ones": np.ones((P, NSK), np.float32),
        })

    res = run_bass_kernel_spmd(
        nc, in_maps, core_ids=list(range(N_CORES)), trace=_CFG["trace"]
    )
    kernel.last_result = res

    outp = np.empty((B, S, DK), np.float32)
    for core in range(N_CORES):
        b, p = core // 2, core % 2
        o = res.results[core]["out"]                 # [2048, 64]
        for j in range(NJ):
            outp[b, (2 * j + p) * CH:(2 * j + p + 1) * CH] = \
                o[j * CH:(j + 1) * CH]
    return outp


# revision 9
# speedup vs baseline: 1.0294x; 1.0294x over previous
"""Causal single-head attention (B=4, S=4096, D=512, dk=64) on 8 Trainium2
NeuronCores via Bass/Tile.

Sharding: core c handles batch b = c//2, query parity p = c%2 — the four
512-row query chunks with global chunk index 2j+p, j=0..3.  Work per job j
is uniform across cores (E[j] = 8j+8 key-tiles of 128); causal boundary
differences between parities are handled by per-core 0/1 mask tensors
(data, not program), so a single SPMD program serves all 8 cores.

Per-core pipeline:
  qT = relu(Wq^T x1T + bq)   [64, 2048]   (PE matmul, DVE bias+relu)
  kT = relu(Wk^T x2T + bk)   [64, 4096]
  vT = relu(Wv^T x2T + bv) -> PE-transposed into v_aug tiles [128, 65]
       (column 64 = 1.0 so the PV matmul also accumulates the softmax
        denominator)
  for each key tile t (sk-outer):            scT = kT_tile^T qT  (PSUM)
      attnT = exp(scT / 8)  (ACT, no max-subtraction: scores in [0, ~6])
      diagonal-band jobs multiply by a mask tile (DVE)
      outT[j] += v_aug_tile^T attnT          (PSUM accum over t)
  finalize: PE-transpose outT -> natural layout, divide by denominator
  column (DVE reciprocal + per-partition scalar multiply), DMA out.

Matmul operands are bf16 (PE runs 1 cycle/row; fp32 is 4 and float32r is
SBUF-bandwidth-capped on this toolchain); accumulation is fp32 in PSUM.
"""
import os
import numpy as np
import ml_dtypes

import bass_rust
import concourse.bass as bass
import concourse.tile as tile
from concourse import mybir
from concourse.bass_utils import run_bass_kernel_spmd
from concourse.masks import make_identity

# ---------------------------------------------------------------- constants
P = 128          # partitions / sk tile
D = 512          # model dim
DK = 64          # key dim
S = 4096         # sequence
B = 4            # batch
CH = 512         # sq chunk width (one job)
NJ = 4           # jobs per core
KD = D // P      # k-tiles in the D contraction
NSK = S // P     # sk tiles
SQ = NJ * CH     # q rows per core
N_CORES = 8

F32 = mybir.dt.float32
BF16 = mybir.dt.bfloat16
F32R = mybir.dt.float32r

_DTMAP = {"bf16": BF16, "f32r": F32R, "f32": F32}
_NPMAP = {"bf16": ml_dtypes.bfloat16, "f32r": np.float32, "f32": np.float32}

_CFG = {
    "dt_proj": os.environ.get("K_DT_PROJ", "bf16"),
    "dt_sc": os.environ.get("K_DT_SC", "bf16"),
    "dt_pv": os.environ.get("K_DT_PV", "bf16"),
    "relu_dve": os.environ.get("K_RELU_DVE", "1") == "1",
    "trace": os.environ.get("K_TRACE", "0") == "1",
}


# ------------------------------------------------- walrus codegen workarounds
def _patch_tile_drain():
    """This neuronxcc rejects >1 sync wait on a CTRL (Drain) instruction;
    TileContext's tail drain carries one wait per live semaphore.  Split the
    waits onto dedicated SP nops, one wait each."""
    from concourse.tile import TileContext

    if getattr(TileContext, "_drain_patched", False):
        return

    def _patched(self, tick_clock, wait_clock):
        nc = self.nc
        probe = nc.sync.nop(nofuse=True, hint="tail_wait_probe")
        wait_clock.add_sem_waits(
            probe.ins, bass_rust.ScopedClock({None: tick_clock.global_clock})
        )
        si = probe.ins.sync_info
        waits = list(si.on_wait) if si is not None else []
        probe.ins.sync_info = bass_rust.SyncInfo(on_wait=waits[:1], on_update=[])
        for w in waits[1:]:
            carrier = nc.sync.nop(nofuse=True, hint="tail_wait")
            carrier.ins.sync_info = bass_rust.SyncInfo(on_wait=[w], on_update=[])
        nc.sync.drain()

        nc.all_engine_barrier()
        assert self.sems is not None
        popped = nc._tile_sem_poison_stack.pop()
        assert popped is self._sem_poison
        nc.clear_and_free_semaphores(list(self.sems.allocated().values()))
        nc.all_engine_barrier()

    TileContext._drain_and_barrier = _patched
    TileContext._drain_patched = True


def _split_sync_waits(nc, max_waits: int = 1):
    """walrus here rejects >1 sync wait on at least CTRL and S3_LW (weight
    load) instruction structs.  Hoist excess waits onto same-engine NOPs
    placed immediately before the instruction (engine streams execute block
    order, so the waits still gate the instruction)."""
    counter = [0]
    for fn in nc.m.functions:
        for bb in fn.blocks:
            changed = False
            new = []
            for inst in bb.instructions:
                si = inst.sync_info
                waits = list(si.on_wait) if si is not None else []
                if len(waits) > max_waits:
                    changed = True
                    for w in waits[:-max_waits]:
                        counter[0] += 1
                        nop = bass_rust.InstNoOp(
                            name=f"I-waitsplit-{counter[0]}", engine=inst.engine
                        )
                        nop.bass_nofuse = True
                        nop.sync_info = bass_rust.SyncInfo(
                            on_wait=[w], on_update=[]
                        )
                        new.append(nop)
                    inst.sync_info = bass_rust.SyncInfo(
                        on_wait=waits[-max_waits:], on_update=list(si.on_update)
                    )
                new.append(inst)
            if changed:
                bb.instructions = new


# ---------------------------------------------------------------- program
def _build_program(causal: bool):
    _patch_tile_drain()
    nc = bass.Bass()

    DT_X = _DTMAP[_CFG["dt_proj"]]
    DT_QK = _DTMAP[_CFG["dt_sc"]]
    DT_AT = _DTMAP[_CFG["dt_pv"]]

    x1t = nc.declare_dram_parameter("x1t", [D, SQ], DT_X, isOutput=False)
    x2t = nc.declare_dram_parameter("x2t", [D, S], DT_X, isOutput=False)
    wq = nc.declare_dram_parameter("wq", [D, DK], DT_X, isOutput=False)
    wk = nc.declare_dram_parameter("wk", [D, DK], DT_X, isOutput=False)
    wv = nc.declare_dram_parameter("wv", [D, DK], DT_X, isOutput=False)
    bq = nc.declare_dram_parameter("bq", [DK], F32, isOutput=False)
    bk = nc.declare_dram_parameter("bk", [DK], F32, isOutput=False)
    bv = nc.declare_dram_parameter("bv", [DK], F32, isOutput=False)
    masks = nc.declare_dram_parameter("masks", [8, P, CH], DT_AT, isOutput=False)
    ones = nc.declare_dram_parameter("ones", [P, NSK], DT_AT, isOutput=False)
    out = nc.declare_dram_parameter("out", [SQ, DK], F32, isOutput=True)

    E = [8 * j + 8 for j in range(NJ)] if causal else [NSK] * NJ

    Relu = mybir.ActivationFunctionType.Relu
    Exp = mybir.ActivationFunctionType.Exp

    def bias_relu(dst, src_psum, bias_sb):
        """dst = relu(src + bias), bias per-partition [p,1]."""
        if _CFG["relu_dve"]:
            nc.vector.tensor_scalar(
                dst, src_psum, bias_sb, 0.0,
                mybir.AluOpType.add, mybir.AluOpType.max,
            )
        else:
            nc.scalar.activation(out=dst, in_=src_psum, func=Relu,
                                 bias=bias_sb, scale=1.0)

    with tile.TileContext(nc) as tc:
        with (
            tc.tile_pool(name="const", bufs=1) as const,
            tc.tile_pool(name="xin", bufs=3) as xin,
            tc.tile_pool(name="resident", bufs=1) as res,
            tc.tile_pool(name="attn", bufs=4) as attn,
            tc.tile_pool(name="ostage", bufs=4) as ostage,
            tc.tile_pool(name="ps", bufs=2, space="PSUM") as ps,
            tc.tile_pool(name="outps", bufs=1, space="PSUM") as outps,
        ):
            # ---------------- constants (weights/biases first; masks late)
            wq_sb = const.tile([P, KD, DK], DT_X)
            wk_sb = const.tile([P, KD, DK], DT_X)
            wv_sb = const.tile([P, KD, DK], DT_X)
            nc.sync.dma_start(out=wq_sb, in_=wq.rearrange("(kd p) m -> p kd m", p=P))
            nc.sync.dma_start(out=wk_sb, in_=wk.rearrange("(kd p) m -> p kd m", p=P))
            nc.sync.dma_start(out=wv_sb, in_=wv.rearrange("(kd p) m -> p kd m", p=P))
            bq_sb = const.tile([DK, 1], F32)
            bk_sb = const.tile([DK, 1], F32)
            bv_sb = const.tile([DK, 1], F32)
            nc.sync.dma_start(out=bq_sb, in_=bq.rearrange("(p o) -> p o", o=1))
            nc.sync.dma_start(out=bk_sb, in_=bk.rearrange("(p o) -> p o", o=1))
            nc.sync.dma_start(out=bv_sb, in_=bv.rearrange("(p o) -> p o", o=1))
            identv = const.tile([P, P], DT_X)
            make_identity(nc, identv)
            ident = const.tile([P, P], F32)
            make_identity(nc, ident)

            # ---------------- qT projection: [64, 2048]
            qT_sb = res.tile([DK, SQ], DT_QK)
            x1v = x1t.rearrange("(kd p) s -> p kd s", p=P)
            for ch in range(SQ // CH):
                xt = xin.tile([P, KD, CH], DT_X, tag="x1c")
                nc.sync.dma_start(out=xt, in_=x1v[:, :, ch * CH:(ch + 1) * CH])
                pq = ps.tile([DK, CH], F32, tag="ps")
                for kd in range(KD):
                    nc.tensor.matmul(
                        pq, wq_sb[:, kd, :], xt[:, kd, :],
                        start=(kd == 0), stop=(kd == KD - 1),
                    )
                bias_relu(qT_sb[:, ch * CH:(ch + 1) * CH], pq, bq_sb)

            # ---------------- kT / v projections over full S
            kT_sb = res.tile([DK, S], DT_QK)
            v_sb = res.tile([P, NSK, DK + 1], DT_AT)
            nc.sync.dma_start(
                out=v_sb[:, :, DK:DK + 1],
                in_=ones.rearrange("p (n o) -> p n o", o=1),
            )
            x2v = x2t.rearrange("(kd p) s -> p kd s", p=P)
            for ch in range(S // CH):
                xt = xin.tile([P, KD, CH], DT_X, tag="x2c")
                nc.sync.dma_start(out=xt, in_=x2v[:, :, ch * CH:(ch + 1) * CH])
                pk = ps.tile([DK, CH], F32, tag="ps")
                for kd in range(KD):
                    nc.tensor.matmul(
                        pk, wk_sb[:, kd, :], xt[:, kd, :],
                        start=(kd == 0), stop=(kd == KD - 1),
                    )
                bias_relu(kT_sb[:, ch * CH:(ch + 1) * CH], pk, bk_sb)
                pv = ps.tile([DK, CH], F32, tag="ps")
                for kd in range(KD):
                    nc.tensor.matmul(
                        pv, wv_sb[:, kd, :], xt[:, kd, :],
                        start=(kd == 0), stop=(kd == KD - 1),
                    )
                vt = attn.tile([DK, CH], DT_X, tag="vT")
                bias_relu(vt, pv, bv_sb)
                for blk in range(CH // P):
                    st = ch * (CH // P) + blk
                    pt = ps.tile([P, DK], DT_X, tag="ps")
                    nc.tensor.transpose(
                        pt, in_=vt[:, blk * P:(blk + 1) * P],
                        identity=identv[:DK, :DK],
                    )
                    nc.vector.tensor_copy(v_sb[:, st, 0:DK], pt)

            # masks arrive late so they don't delay the projection DMAs
            if causal:
                masks_sb = const.tile([P, 8, CH], DT_AT)
                nc.sync.dma_start(out=masks_sb, in_=masks.rearrange("m p s -> p m s"))

            # ---------------- attention, sk-outer
            outT = [
                outps.tile([DK + 1, CH], F32, name=f"outT{j}", tag=f"outT{j}")
                for j in range(NJ)
            ]
            for t in range(NSK):
                jobs = [j for j in range(NJ) if E[j] > t]
                kslc = kT_sb[:, t * P:(t + 1) * P]
                # pair jobs into [128, 1024] psum tiles to halve ACT overhead
                for i0 in range(0, len(jobs), 2):
                    pair = jobs[i0:i0 + 2]
                    w = len(pair) * CH
                    sc = ps.tile([P, 1024], F32, tag="ps")
                    at = attn.tile([P, 1024], DT_AT, tag="attnT")
                    for idx, j in enumerate(pair):
                        nc.tensor.matmul(
                            sc[:, idx * CH:(idx + 1) * CH],
                            kslc,
                            qT_sb[:, j * CH:(j + 1) * CH],
                            start=True,
                            stop=True,
                        )
                    nc.scalar.activation(
                        out=at[:, :w], in_=sc[:, :w], func=Exp, scale=0.125
                    )
                    for idx, j in enumerate(pair):
                        aslc = at[:, idx * CH:(idx + 1) * CH]
                        if causal and j == t // 8:
                            nc.vector.tensor_tensor(
                                aslc, aslc, masks_sb[:, t % 8, :],
                                mybir.AluOpType.mult,
                            )
                        nc.tensor.matmul(
                            outT[j],
                            v_sb[:, t, :],
                            aslc,
                            start=(t == 0),
                            stop=(t == E[j] - 1),
                            skip_group_check=True,
                        )
                # finalize completed jobs
                for j in range(NJ):
                    if E[j] - 1 != t:
                        continue
                    oT = ostage.tile([DK + 1, CH], F32, tag="oT")
                    nc.vector.tensor_copy(oT, outT[j])
                    for blk in range(CH // P):
                        po = ps.tile([P, DK + 1], F32, tag="ps")
                        nc.tensor.transpose(
                            po,
                            in_=oT[:, blk * P:(blk + 1) * P],
                            identity=ident[:DK + 1, :DK + 1],
                        )
                        rec = ostage.tile([P, 1], F32, tag="rec")
                        nc.vector.reciprocal(rec, po[:, DK:DK + 1])
                        ot = ostage.tile([P, DK], F32, tag="ot")
                        nc.vector.tensor_scalar_mul(ot, po[:, 0:DK], rec)
                        r0 = j * CH + blk * P
                        nc.sync.dma_start(out=out[r0:r0 + P, :], in_=ot)

    _split_sync_waits(nc)
    return nc


_PROGRAMS = {}


def _program(causal: bool):
    if causal not in _PROGRAMS:
        _PROGRAMS[causal] = _build_program(causal)
    return _PROGRAMS[causal]


def _host_masks(parity: int) -> np.ndarray:
    """masks[m] multiplies the exp'd [sk=128, sq=512] tile of the job whose
    diagonal band covers key tiles [E-8, E); m = position in that band."""
    sk = np.arange(P)[:, None]
    sq = np.arange(CH)[None, :]
    m = np.zeros((8, P, CH), np.float32)
    for i in range(8):
        if parity == 1:
            if i < 4:
                m[i] = 1.0
            else:
                r = i - 4
                m[i] = (sq >= r * P + sk).astype(np.float32)
        else:
            if i < 4:
                m[i] = (sq >= i * P + sk).astype(np.float32)
            else:
                m[i] = 0.0
    return m


def kernel(x1, x2, Wq, bq, Wk, bk, Wv, bv, apply_mask):
    np_x = _NPMAP[_CFG["dt_proj"]]
    np_at = _NPMAP[_CFG["dt_pv"]]
    x1 = np.asarray(x1, dtype=np.float32)
    x2 = np.asarray(x2, dtype=np.float32)
    Wq_h = np.ascontiguousarray(np.asarray(Wq, np.float32).astype(np_x))
    Wk_h = np.ascontiguousarray(np.asarray(Wk, np.float32).astype(np_x))
    Wv_h = np.ascontiguousarray(np.asarray(Wv, np.float32).astype(np_x))
    bq_h = np.ascontiguousarray(np.asarray(bq, np.float32))
    bk_h = np.ascontiguousarray(np.asarray(bk, np.float32))
    bv_h = np.ascontiguousarray(np.asarray(bv, np.float32))
    causal = bool(int(np.asarray(apply_mask)))

    nc = _program(causal)

    x2t_h = [np.ascontiguousarray(x2[b].T).astype(np_x) for b in range(B)]
    ones_h = np.ones((P, NSK), np_at)
    masks_h = [_host_masks(p).astype(np_at) for p in range(2)]

    in_maps = []
    for core in range(N_CORES):
        b, p = core // 2, core % 2
        xb = x1[b]                                   # [S, D]
        rows = np.concatenate(
            [xb[(2 * j + p) * CH:(2 * j + p + 1) * CH] for j in range(NJ)], axis=0
        )                                            # [2048, D]
        in_maps.append({
            "x1t": np.ascontiguousarray(rows.T).astype(np_x),   # [D, 2048]
            "x2t": x2t_h[b],                                    # [D, 4096]
            "wq": Wq_h, "wk": Wk_h, "wv": Wv_h,
            "bq": bq_h, "bk": bk_h, "bv": bv_h,
            "masks": masks_h[p],
            "ones": ones_h,
        })

    res = run_bass_kernel_spmd(
        nc, in_maps, core_ids=list(range(N_CORES)), trace=_CFG["trace"]
    )
    kernel.last_result = res

    outp = np.empty((B, S, DK), np.float32)
    for core in range(N_CORES):
        b, p = core // 2, core % 2
        o = res.results[core]["out"]                 # [2048, 64]
        for j in range(NJ):
            outp[b, (2 * j + p) * CH:(2 * j + p + 1) * CH] = \
                o[j * CH:(j + 1) * CH]
    return outp


# revision 10
# speedup vs baseline: 1.2550x; 1.2191x over previous
"""Causal single-head attention (B=4, S=4096, D=512, dk=64) on 8 Trainium2
NeuronCores via Bass/Tile.

Sharding: core c handles batch b = c//2, query parity p = c%2 — the four
512-row query chunks with global chunk index 2j+p, j=0..3.  Work per job j
is uniform across cores (E[j] = 8j+8 key-tiles of 128); causal boundary
differences between parities are handled by per-core 0/1 mask tensors
(data, not program), so a single SPMD program serves all 8 cores.

Per-core pipeline:
  qT = relu(Wq^T x1T + bq)   [64, 2048]   (PE matmul, DVE bias+relu)
  kT = relu(Wk^T x2T + bk)   [64, 4096]
  vT = relu(Wv^T x2T + bv) -> PE-transposed into v_aug tiles [128, 65]
       (column 64 = 1.0 so the PV matmul also accumulates the softmax
        denominator)
  for each key tile t (sk-outer, PV deferred one tile so the PE stream
  never waits on the exp of the scores it just produced):
      scT(t) = kT_tile^T qT               (PE -> PSUM)
      attnT(t) = exp(scT(t) / 8)          (ACT; no max-subtraction —
                                           scores are in [0, ~6])
      diagonal-band job multiplies attnT by a mask tile (GpSimd)
      outT[j] += v_aug(t-1)^T attnT(t-1)  (PE, PSUM accum)
  finalize: PE-transpose outT -> natural layout, divide by the
  denominator column (DVE reciprocal + per-partition multiply), DMA out.

Matmul operands are bf16 (PE runs 1 cycle/row; fp32 is 4 and float32r is
SBUF-bandwidth-capped on this toolchain); accumulation is fp32 in PSUM.
"""
import os
import numpy as np
import ml_dtypes

import bass_rust
import concourse.bass as bass
import concourse.tile as tile
from concourse import mybir
from concourse.bass_utils import run_bass_kernel_spmd
from concourse.masks import make_identity

# ---------------------------------------------------------------- constants
P = 128          # partitions / sk tile
D = 512          # model dim
DK = 64          # key dim
S = 4096         # sequence
B = 4            # batch
CH = 512         # sq chunk width (one job)
NJ = 4           # jobs per core
KD = D // P      # k-tiles in the D contraction
NSK = S // P     # sk tiles
SQ = NJ * CH     # q rows per core
N_CORES = 8

F32 = mybir.dt.float32
BF16 = mybir.dt.bfloat16
F32R = mybir.dt.float32r

_DTMAP = {"bf16": BF16, "f32r": F32R, "f32": F32}
_NPMAP = {"bf16": ml_dtypes.bfloat16, "f32r": np.float32, "f32": np.float32}

_CFG = {
    "dt_proj": os.environ.get("K_DT_PROJ", "bf16"),
    "dt_sc": os.environ.get("K_DT_SC", "bf16"),
    "dt_pv": os.environ.get("K_DT_PV", "bf16"),
    "relu_dve": os.environ.get("K_RELU_DVE", "1") == "1",
    "mask_pool": os.environ.get("K_MASK_POOL", "1") == "1",
    "trace": os.environ.get("K_TRACE", "0") == "1",
}


# ------------------------------------------------- walrus codegen workarounds
def _patch_tile_drain():
    """This neuronxcc rejects >1 sync wait on a CTRL (Drain) instruction;
    TileContext's tail drain carries one wait per live semaphore.  Split the
    waits onto dedicated SP nops, one wait each."""
    from concourse.tile import TileContext

    if getattr(TileContext, "_drain_patched", False):
        return

    def _patched(self, tick_clock, wait_clock):
        nc = self.nc
        probe = nc.sync.nop(nofuse=True, hint="tail_wait_probe")
        wait_clock.add_sem_waits(
            probe.ins, bass_rust.ScopedClock({None: tick_clock.global_clock})
        )
        si = probe.ins.sync_info
        waits = list(si.on_wait) if si is not None else []
        probe.ins.sync_info = bass_rust.SyncInfo(on_wait=waits[:1], on_update=[])
        for w in waits[1:]:
            carrier = nc.sync.nop(nofuse=True, hint="tail_wait")
            carrier.ins.sync_info = bass_rust.SyncInfo(on_wait=[w], on_update=[])
        nc.sync.drain()

        nc.all_engine_barrier()
        assert self.sems is not None
        popped = nc._tile_sem_poison_stack.pop()
        assert popped is self._sem_poison
        nc.clear_and_free_semaphores(list(self.sems.allocated().values()))
        nc.all_engine_barrier()

    TileContext._drain_and_barrier = _patched
    TileContext._drain_patched = True


def _split_sync_waits(nc, max_waits: int = 1):
    """walrus here rejects >1 sync wait on at least CTRL and S3_LW (weight
    load) instruction structs.  Hoist excess waits onto same-engine NOPs
    placed immediately before the instruction (engine streams execute block
    order, so the waits still gate the instruction)."""
    counter = [0]
    for fn in nc.m.functions:
        for bb in fn.blocks:
            changed = False
            new = []
            for inst in bb.instructions:
                si = inst.sync_info
                waits = list(si.on_wait) if si is not None else []
                if len(waits) > max_waits:
                    changed = True
                    for w in waits[:-max_waits]:
                        counter[0] += 1
                        nop = bass_rust.InstNoOp(
                            name=f"I-waitsplit-{counter[0]}", engine=inst.engine
                        )
                        nop.bass_nofuse = True
                        nop.sync_info = bass_rust.SyncInfo(
                            on_wait=[w], on_update=[]
                        )
                        new.append(nop)
                    inst.sync_info = bass_rust.SyncInfo(
                        on_wait=waits[-max_waits:], on_update=list(si.on_update)
                    )
                new.append(inst)
            if changed:
                bb.instructions = new


# ---------------------------------------------------------------- program
def _build_program(causal: bool):
    _patch_tile_drain()
    nc = bass.Bass()

    DT_X = _DTMAP[_CFG["dt_proj"]]
    DT_QK = _DTMAP[_CFG["dt_sc"]]
    DT_AT = _DTMAP[_CFG["dt_pv"]]

    # chunk-contiguous host layouts: one DMA per 512-column chunk, each a
    # fully contiguous [128, KD*CH] block
    x1c = nc.declare_dram_parameter("x1c", [SQ // CH, P, KD * CH], DT_X,
                                    isOutput=False)
    x2c = nc.declare_dram_parameter("x2c", [S // CH, P, KD * CH], DT_X,
                                    isOutput=False)
    wq = nc.declare_dram_parameter("wq", [D, DK], DT_X, isOutput=False)
    wk = nc.declare_dram_parameter("wk", [D, DK], DT_X, isOutput=False)
    wv = nc.declare_dram_parameter("wv", [D, DK], DT_X, isOutput=False)
    bq = nc.declare_dram_parameter("bq", [DK], F32, isOutput=False)
    bk = nc.declare_dram_parameter("bk", [DK], F32, isOutput=False)
    bv = nc.declare_dram_parameter("bv", [DK], F32, isOutput=False)
    masks = nc.declare_dram_parameter("masks", [8, P, CH], DT_AT, isOutput=False)
    ones = nc.declare_dram_parameter("ones", [P, NSK], DT_AT, isOutput=False)
    out = nc.declare_dram_parameter("out", [SQ, DK], F32, isOutput=True)

    E = [8 * j + 8 for j in range(NJ)] if causal else [NSK] * NJ

    Exp = mybir.ActivationFunctionType.Exp
    Relu = mybir.ActivationFunctionType.Relu

    def bias_relu(dst, src_psum, bias_sb):
        """dst = relu(src + bias), bias per-partition [p,1]."""
        if _CFG["relu_dve"]:
            nc.vector.tensor_scalar(
                dst, src_psum, bias_sb, 0.0,
                mybir.AluOpType.add, mybir.AluOpType.max,
            )
        else:
            nc.scalar.activation(out=dst, in_=src_psum, func=Relu,
                                 bias=bias_sb, scale=1.0)

    with tile.TileContext(nc) as tc:
        with (
            tc.tile_pool(name="const", bufs=1) as const,
            tc.tile_pool(name="xin", bufs=4) as xin,
            tc.tile_pool(name="resident", bufs=1) as res,
            tc.tile_pool(name="attn", bufs=4) as attn,
            tc.tile_pool(name="ostage", bufs=4) as ostage,
            tc.tile_pool(name="outps", bufs=1, space="PSUM") as outps,
        ):
            # outT accumulators live across the whole attention loop: 4 banks
            outT = [
                outps.tile([DK + 1, CH], F32, name=f"outT{j}", tag=f"outT{j}")
                for j in range(NJ)
            ]

            # ---------------- constants (weights/biases first; masks late)
            wq_sb = const.tile([P, KD, DK], DT_X)
            wk_sb = const.tile([P, KD, DK], DT_X)
            wv_sb = const.tile([P, KD, DK], DT_X)
            nc.sync.dma_start(out=wq_sb, in_=wq.rearrange("(kd p) m -> p kd m", p=P))
            nc.sync.dma_start(out=wk_sb, in_=wk.rearrange("(kd p) m -> p kd m", p=P))
            nc.sync.dma_start(out=wv_sb, in_=wv.rearrange("(kd p) m -> p kd m", p=P))
            bq_sb = const.tile([DK, 1], F32)
            bk_sb = const.tile([DK, 1], F32)
            bv_sb = const.tile([DK, 1], F32)
            nc.sync.dma_start(out=bq_sb, in_=bq.rearrange("(p o) -> p o", o=1))
            nc.sync.dma_start(out=bk_sb, in_=bk.rearrange("(p o) -> p o", o=1))
            nc.sync.dma_start(out=bv_sb, in_=bv.rearrange("(p o) -> p o", o=1))
            identv = const.tile([P, P], DT_X)
            make_identity(nc, identv)
            ident = const.tile([P, P], F32)
            make_identity(nc, ident)

            qT_sb = res.tile([DK, SQ], DT_QK)
            kT_sb = res.tile([DK, S], DT_QK)
            v_sb = res.tile([P, NSK, DK + 1], DT_AT)
            nc.sync.dma_start(
                out=v_sb[:, :, DK:DK + 1],
                in_=ones.rearrange("p (n o) -> p n o", o=1),
            )

            # ---------------- projections (own PSUM pool, closed before
            # the attention pool opens — stack allocation reuses the banks)
            with tc.tile_pool(name="pps", bufs=4, space="PSUM") as pps:
                for ch in range(SQ // CH):
                    xt = xin.tile([P, KD, CH], DT_X, tag="x1c")
                    nc.sync.dma_start(
                        out=xt,
                        in_=x1c[ch].rearrange("p (kd s) -> p kd s", kd=KD),
                    )
                    pq = pps.tile([DK, CH], F32, tag="pps")
                    for kd in range(KD):
                        nc.tensor.matmul(
                            pq, wq_sb[:, kd, :], xt[:, kd, :],
                            start=(kd == 0), stop=(kd == KD - 1),
                        )
                    bias_relu(qT_sb[:, ch * CH:(ch + 1) * CH], pq, bq_sb)

                for ch in range(S // CH):
                    xt = xin.tile([P, KD, CH], DT_X, tag="x2c")
                    nc.sync.dma_start(
                        out=xt,
                        in_=x2c[ch].rearrange("p (kd s) -> p kd s", kd=KD),
                    )
                    pk = pps.tile([DK, CH], F32, tag="pps")
                    for kd in range(KD):
                        nc.tensor.matmul(
                            pk, wk_sb[:, kd, :], xt[:, kd, :],
                            start=(kd == 0), stop=(kd == KD - 1),
                        )
                    bias_relu(kT_sb[:, ch * CH:(ch + 1) * CH], pk, bk_sb)
                    pv = pps.tile([DK, CH], F32, tag="pps")
                    for kd in range(KD):
                        nc.tensor.matmul(
                            pv, wv_sb[:, kd, :], xt[:, kd, :],
                            start=(kd == 0), stop=(kd == KD - 1),
                        )
                    vt = attn.tile([DK, CH], DT_X, tag="vT")
                    bias_relu(vt, pv, bv_sb)
                    for blk in range(CH // P):
                        st = ch * (CH // P) + blk
                        pt = pps.tile([P, DK], DT_X, tag="pps")
                        nc.tensor.transpose(
                            pt, in_=vt[:, blk * P:(blk + 1) * P],
                            identity=identv[:DK, :DK],
                        )
                        nc.vector.tensor_copy(v_sb[:, st, 0:DK], pt)

            # masks arrive late so they don't delay the projection DMAs
            if causal:
                masks_sb = const.tile([P, 8, CH], DT_AT)
                nc.sync.dma_start(out=masks_sb, in_=masks.rearrange("m p s -> p m s"))

            # ---------------- attention, sk-outer, PV deferred one tile
            with tc.tile_pool(name="sps", bufs=2, space="PSUM") as sps:
                pending = []     # (j, attnT slice, start, stop) from tile t-1
                for t in range(NSK + 1):
                    if t < NSK:
                        jobs = [j for j in range(NJ) if E[j] > t]
                        kslc = kT_sb[:, t * P:(t + 1) * P]
                        new_pending = []
                        for i0 in range(0, len(jobs), 2):
                            pair = jobs[i0:i0 + 2]
                            w = len(pair) * CH
                            sc = sps.tile([P, 1024], F32, tag="sc")
                            at = attn.tile([P, 1024], DT_AT, tag="attnT")
                            for idx, j in enumerate(pair):
                                nc.tensor.matmul(
                                    sc[:, idx * CH:(idx + 1) * CH],
                                    kslc,
                                    qT_sb[:, j * CH:(j + 1) * CH],
                                    start=True,
                                    stop=True,
                                )
                            nc.scalar.activation(
                                out=at[:, :w], in_=sc[:, :w], func=Exp,
                                scale=0.125,
                            )
                            for idx, j in enumerate(pair):
                                aslc = at[:, idx * CH:(idx + 1) * CH]
                                if causal and j == t // 8:
                                    eng = (nc.gpsimd if _CFG["mask_pool"]
                                           else nc.vector)
                                    eng.tensor_tensor(
                                        aslc, aslc, masks_sb[:, t % 8, :],
                                        mybir.AluOpType.mult,
                                    )
                                new_pending.append(
                                    (j, aslc, t == 0, t == E[j] - 1)
                                )
                    else:
                        new_pending = []

                    # PV for tile t-1 (runs while ACT exps tile t)
                    for j, aslc, st_flag, sp_flag in pending:
                        nc.tensor.matmul(
                            outT[j],
                            v_sb[:, t - 1, :],
                            aslc,
                            start=st_flag,
                            stop=sp_flag,
                            skip_group_check=True,
                        )
                    pending = new_pending

                    # finalize jobs whose last PV was just emitted (E[j]==t)
                    for j in range(NJ):
                        if E[j] != t:
                            continue
                        oT = ostage.tile([DK + 1, CH], F32, tag="oT")
                        nc.vector.tensor_copy(oT, outT[j])
                        for blk in range(CH // P):
                            po = sps.tile([P, DK + 1], F32, tag="sc")
                            nc.tensor.transpose(
                                po,
                                in_=oT[:, blk * P:(blk + 1) * P],
                                identity=ident[:DK + 1, :DK + 1],
                            )
                            rec = ostage.tile([P, 1], F32, tag="rec")
                            nc.vector.reciprocal(rec, po[:, DK:DK + 1])
                            ot = ostage.tile([P, DK], F32, tag="ot")
                            nc.vector.tensor_scalar_mul(ot, po[:, 0:DK], rec)
                            r0 = j * CH + blk * P
                            nc.sync.dma_start(out=out[r0:r0 + P, :], in_=ot)

    _split_sync_waits(nc)
    return nc


_PROGRAMS = {}


def _program(causal: bool):
    if causal not in _PROGRAMS:
        _PROGRAMS[causal] = _build_program(causal)
    return _PROGRAMS[causal]


def _host_masks(parity: int) -> np.ndarray:
    """masks[m] multiplies the exp'd [sk=128, sq=512] tile of the job whose
    diagonal band covers key tiles [E-8, E); m = position in that band."""
    sk = np.arange(P)[:, None]
    sq = np.arange(CH)[None, :]
    m = np.zeros((8, P, CH), np.float32)
    for i in range(8):
        if parity == 1:
            if i < 4:
                m[i] = 1.0
            else:
                r = i - 4
                m[i] = (sq >= r * P + sk).astype(np.float32)
        else:
            if i < 4:
                m[i] = (sq >= i * P + sk).astype(np.float32)
            else:
                m[i] = 0.0
    return m


def _chunked(xt_rows: np.ndarray, np_x) -> np.ndarray:
    """[rows, D] -> [nch, 128, KD*CH] where [ch, p, kd*CH+s] =
    x[ch*CH+s, kd*128+p]."""
    nch = xt_rows.shape[0] // CH
    a = xt_rows.reshape(nch, CH, KD, P).transpose(0, 3, 2, 1)
    return np.ascontiguousarray(a.reshape(nch, P, KD * CH).astype(np_x))


def kernel(x1, x2, Wq, bq, Wk, bk, Wv, bv, apply_mask):
    np_x = _NPMAP[_CFG["dt_proj"]]
    np_at = _NPMAP[_CFG["dt_pv"]]
    x1 = np.asarray(x1, dtype=np.float32)
    x2 = np.asarray(x2, dtype=np.float32)
    Wq_h = np.ascontiguousarray(np.asarray(Wq, np.float32).astype(np_x))
    Wk_h = np.ascontiguousarray(np.asarray(Wk, np.float32).astype(np_x))
    Wv_h = np.ascontiguousarray(np.asarray(Wv, np.float32).astype(np_x))
    bq_h = np.ascontiguousarray(np.asarray(bq, np.float32))
    bk_h = np.ascontiguousarray(np.asarray(bk, np.float32))
    bv_h = np.ascontiguousarray(np.asarray(bv, np.float32))
    causal = bool(int(np.asarray(apply_mask)))

    nc = _program(causal)

    x2c_h = [_chunked(x2[b], np_x) for b in range(B)]
    ones_h = np.ones((P, NSK), np_at)
    masks_h = [_host_masks(p).astype(np_at) for p in range(2)]

    in_maps = []
    for core in range(N_CORES):
        b, p = core // 2, core % 2
        xb = x1[b]                                   # [S, D]
        rows = np.concatenate(
            [xb[(2 * j + p) * CH:(2 * j + p + 1) * CH] for j in range(NJ)], axis=0
        )                                            # [2048, D]
        in_maps.append({
            "x1c": _chunked(rows, np_x),
            "x2c": x2c_h[b],
            "wq": Wq_h, "wk": Wk_h, "wv": Wv_h,
            "bq": bq_h, "bk": bk_h, "bv": bv_h,
            "masks": masks_h[p],
            "ones": ones_h,
        })

    res = run_bass_kernel_spmd(
        nc, in_maps, core_ids=list(range(N_CORES)), trace=_CFG["trace"]
    )
    kernel.last_result = res

    outp = np.empty((B, S, DK), np.float32)
    for core in range(N_CORES):
        b, p = core // 2, core % 2
        o = res.results[core]["out"]                 # [2048, 64]
        for j in range(NJ):
            outp[b, (2 * j + p) * CH:(2 * j + p + 1) * CH] = \
                o[j * CH:(j + 1) * CH]
    return outp


# revision 11
# speedup vs baseline: 1.3460x; 1.0725x over previous
"""Causal single-head attention (B=4, S=4096, D=512, dk=64) on 8 Trainium2
NeuronCores via Bass/Tile.

Sharding: core c handles batch b = c//2, query parity p = c%2 — the four
512-row query chunks with global chunk index 2j+p, j=0..3.  Work per job j
is uniform across cores (E[j] = 8j+8 key-tiles of 128); causal boundary
differences between parities are handled by per-core 0/1 mask tensors
(data, not program), so a single SPMD program serves all 8 cores.

Per-core pipeline (emission interleaves projection chunks with attention
jobs so the in-order PE stream stays dense and the HAM clock-gate keeps
the PE at 2.4 GHz):

  group j:  project qT chunk j;  project kT/vT for x2 chunks 2j, 2j+1;
            PE-transpose vT tiles into v_aug [128, 65] tiles (column 64
            is 1.0 so the PV matmul also accumulates the softmax
            denominator);  then run attention job j over key tiles
            t < E[j], two tiles per scores-PSUM buffer:
               scT(t,t+1) = kT_tile^T qT_j          (PE -> PSUM pair)
               attnT = exp(scT / 8)                 (ACT; no max needed,
                                                     scores in [0, ~6])
               diagonal-band tiles multiply by a mask tile (DVE)
               outT[j] += v_aug(t)^T attnT(t)       (PE, deferred one
                                                     pair so PE never
                                                     waits on exp)
            finalize: PE-transpose outT, divide by denominator column
            (DVE reciprocal + per-partition multiply), DMA out.

Matmul operands are bf16 (PE runs 1 cycle/row; fp32 is 4 and float32r is
SBUF-bandwidth-capped on this toolchain); accumulation is fp32 in PSUM.
"""
import os
import numpy as np
import ml_dtypes

import bass_rust
import concourse.bass as bass
import concourse.tile as tile
from concourse import mybir
from concourse.bass_utils import run_bass_kernel_spmd
from concourse.masks import make_identity

# ---------------------------------------------------------------- constants
P = 128          # partitions / sk tile
D = 512          # model dim
DK = 64          # key dim
S = 4096         # sequence
B = 4            # batch
CH = 512         # sq chunk width (one job)
NJ = 4           # jobs per core
KD = D // P      # k-tiles in the D contraction
NSK = S // P     # sk tiles
SQ = NJ * CH     # q rows per core
N_CORES = 8

F32 = mybir.dt.float32
BF16 = mybir.dt.bfloat16
F32R = mybir.dt.float32r

_DTMAP = {"bf16": BF16, "f32r": F32R, "f32": F32}
_NPMAP = {"bf16": ml_dtypes.bfloat16, "f32r": np.float32, "f32": np.float32}

_CFG = {
    "dt_proj": os.environ.get("K_DT_PROJ", "bf16"),
    "dt_sc": os.environ.get("K_DT_SC", "bf16"),
    "dt_pv": os.environ.get("K_DT_PV", "bf16"),
    "relu_dve": os.environ.get("K_RELU_DVE", "0") == "1",
    "mask_pool": os.environ.get("K_MASK_POOL", "0") == "1",
    "trace": os.environ.get("K_TRACE", "0") == "1",
}


# ------------------------------------------------- walrus codegen workarounds
def _patch_tile_drain():
    """This neuronxcc rejects >1 sync wait on a CTRL (Drain) instruction;
    TileContext's tail drain carries one wait per live semaphore.  Split the
    waits onto dedicated SP nops, one wait each."""
    from concourse.tile import TileContext

    if getattr(TileContext, "_drain_patched", False):
        return

    def _patched(self, tick_clock, wait_clock):
        nc = self.nc
        probe = nc.sync.nop(nofuse=True, hint="tail_wait_probe")
        wait_clock.add_sem_waits(
            probe.ins, bass_rust.ScopedClock({None: tick_clock.global_clock})
        )
        si = probe.ins.sync_info
        waits = list(si.on_wait) if si is not None else []
        probe.ins.sync_info = bass_rust.SyncInfo(on_wait=waits[:1], on_update=[])
        for w in waits[1:]:
            carrier = nc.sync.nop(nofuse=True, hint="tail_wait")
            carrier.ins.sync_info = bass_rust.SyncInfo(on_wait=[w], on_update=[])
        nc.sync.drain()

        nc.all_engine_barrier()
        assert self.sems is not None
        popped = nc._tile_sem_poison_stack.pop()
        assert popped is self._sem_poison
        nc.clear_and_free_semaphores(list(self.sems.allocated().values()))
        nc.all_engine_barrier()

    TileContext._drain_and_barrier = _patched
    TileContext._drain_patched = True


def _split_sync_waits(nc, max_waits: int = 1):
    """walrus here rejects >1 sync wait on at least CTRL and S3_LW (weight
    load) instruction structs.  Hoist excess waits onto same-engine NOPs
    placed immediately before the instruction (engine streams execute block
    order, so the waits still gate the instruction)."""
    counter = [0]
    for fn in nc.m.functions:
        for bb in fn.blocks:
            changed = False
            new = []
            for inst in bb.instructions:
                si = inst.sync_info
                waits = list(si.on_wait) if si is not None else []
                if len(waits) > max_waits:
                    changed = True
                    for w in waits[:-max_waits]:
                        counter[0] += 1
                        nop = bass_rust.InstNoOp(
                            name=f"I-waitsplit-{counter[0]}", engine=inst.engine
                        )
                        nop.bass_nofuse = True
                        nop.sync_info = bass_rust.SyncInfo(
                            on_wait=[w], on_update=[]
                        )
                        new.append(nop)
                    inst.sync_info = bass_rust.SyncInfo(
                        on_wait=waits[-max_waits:], on_update=list(si.on_update)
                    )
                new.append(inst)
            if changed:
                bb.instructions = new


# ---------------------------------------------------------------- program
def _build_program(causal: bool):
    _patch_tile_drain()
    nc = bass.Bass()

    DT_X = _DTMAP[_CFG["dt_proj"]]
    DT_QK = _DTMAP[_CFG["dt_sc"]]
    DT_AT = _DTMAP[_CFG["dt_pv"]]

    # chunk-contiguous host layouts: one DMA per 512-column chunk, each a
    # fully contiguous [128, KD*CH] block
    x1c = nc.declare_dram_parameter("x1c", [SQ // CH, P, KD * CH], DT_X,
                                    isOutput=False)
    x2c = nc.declare_dram_parameter("x2c", [S // CH, P, KD * CH], DT_X,
                                    isOutput=False)
    wq = nc.declare_dram_parameter("wq", [D, DK], DT_X, isOutput=False)
    wk = nc.declare_dram_parameter("wk", [D, DK], DT_X, isOutput=False)
    wv = nc.declare_dram_parameter("wv", [D, DK], DT_X, isOutput=False)
    bq = nc.declare_dram_parameter("bq", [DK], F32, isOutput=False)
    bk = nc.declare_dram_parameter("bk", [DK], F32, isOutput=False)
    bv = nc.declare_dram_parameter("bv", [DK], F32, isOutput=False)
    masks = nc.declare_dram_parameter("masks", [8, P, CH], DT_AT, isOutput=False)
    ones = nc.declare_dram_parameter("ones", [P, NSK], DT_AT, isOutput=False)
    out = nc.declare_dram_parameter("out", [SQ, DK], F32, isOutput=True)

    E = [8 * j + 8 for j in range(NJ)] if causal else [NSK] * NJ

    Exp = mybir.ActivationFunctionType.Exp
    Relu = mybir.ActivationFunctionType.Relu

    def bias_relu(dst, src_psum, bias_sb):
        """dst = relu(src + bias), bias per-partition [p,1]."""
        if _CFG["relu_dve"]:
            nc.vector.tensor_scalar(
                dst, src_psum, bias_sb, 0.0,
                mybir.AluOpType.add, mybir.AluOpType.max,
            )
        else:
            nc.scalar.activation(out=dst, in_=src_psum, func=Relu,
                                 bias=bias_sb, scale=1.0)

    with tile.TileContext(nc) as tc:
        with (
            tc.tile_pool(name="const", bufs=1) as const,
            tc.tile_pool(name="xin", bufs=4) as xin,
            tc.tile_pool(name="resident", bufs=1) as res,
            tc.tile_pool(name="attn", bufs=4) as attn,
            tc.tile_pool(name="ostage", bufs=4) as ostage,
            tc.tile_pool(name="outps", bufs=2, space="PSUM") as outps,
            tc.tile_pool(name="pps", bufs=2, space="PSUM") as pps,
            tc.tile_pool(name="sps", bufs=2, space="PSUM") as sps,
        ):
            # ---------------- constants
            wq_sb = const.tile([P, KD, DK], DT_X)
            wk_sb = const.tile([P, KD, DK], DT_X)
            wv_sb = const.tile([P, KD, DK], DT_X)
            nc.sync.dma_start(out=wq_sb, in_=wq.rearrange("(kd p) m -> p kd m", p=P))
            nc.sync.dma_start(out=wk_sb, in_=wk.rearrange("(kd p) m -> p kd m", p=P))
            nc.sync.dma_start(out=wv_sb, in_=wv.rearrange("(kd p) m -> p kd m", p=P))
            bq_sb = const.tile([DK, 1], F32)
            bk_sb = const.tile([DK, 1], F32)
            bv_sb = const.tile([DK, 1], F32)
            nc.sync.dma_start(out=bq_sb, in_=bq.rearrange("(p o) -> p o", o=1))
            nc.sync.dma_start(out=bk_sb, in_=bk.rearrange("(p o) -> p o", o=1))
            nc.sync.dma_start(out=bv_sb, in_=bv.rearrange("(p o) -> p o", o=1))
            identv = const.tile([P, P], DT_X)
            make_identity(nc, identv)
            ident = const.tile([P, P], F32)
            make_identity(nc, ident)
            if causal:
                masks_sb = const.tile([P, 8, CH], DT_AT)
                nc.sync.dma_start(out=masks_sb, in_=masks.rearrange("m p s -> p m s"))

            qT_sb = res.tile([DK, SQ], DT_QK)
            kT_sb = res.tile([DK, S], DT_QK)
            vT_sb = res.tile([DK, S], DT_X)
            v_sb = res.tile([P, NSK, DK + 1], DT_AT)
            nc.sync.dma_start(
                out=v_sb[:, :, DK:DK + 1],
                in_=ones.rearrange("p (n o) -> p n o", o=1),
            )

            def proj_q_chunk(ch):
                xt = xin.tile([P, KD, CH], DT_X, tag="x1c")
                nc.sync.dma_start(
                    out=xt, in_=x1c[ch].rearrange("p (kd s) -> p kd s", kd=KD)
                )
                pq = pps.tile([DK, CH], F32, tag="pps")
                for kd in range(KD):
                    nc.tensor.matmul(
                        pq, wq_sb[:, kd, :], xt[:, kd, :],
                        start=(kd == 0), stop=(kd == KD - 1),
                    )
                bias_relu(qT_sb[:, ch * CH:(ch + 1) * CH], pq, bq_sb)

            def proj_kv_chunk(ch):
                xt = xin.tile([P, KD, CH], DT_X, tag="x2c")
                nc.sync.dma_start(
                    out=xt, in_=x2c[ch].rearrange("p (kd s) -> p kd s", kd=KD)
                )
                pk = pps.tile([DK, CH], F32, tag="pps")
                for kd in range(KD):
                    nc.tensor.matmul(
                        pk, wk_sb[:, kd, :], xt[:, kd, :],
                        start=(kd == 0), stop=(kd == KD - 1),
                    )
                bias_relu(kT_sb[:, ch * CH:(ch + 1) * CH], pk, bk_sb)
                pv = pps.tile([DK, CH], F32, tag="pps")
                for kd in range(KD):
                    nc.tensor.matmul(
                        pv, wv_sb[:, kd, :], xt[:, kd, :],
                        start=(kd == 0), stop=(kd == KD - 1),
                    )
                bias_relu(vT_sb[:, ch * CH:(ch + 1) * CH], pv, bv_sb)

            def transpose_v(st):
                pt = pps.tile([P, DK], DT_X, tag="pps")
                nc.tensor.transpose(
                    pt, in_=vT_sb[:, st * P:(st + 1) * P],
                    identity=identv[:DK, :DK],
                )
                nc.vector.tensor_copy(v_sb[:, st, 0:DK], pt)

            def attention_job(j):
                oT_ps = outps.tile([DK + 1, CH], F32, tag="outT")
                qslc = qT_sb[:, j * CH:(j + 1) * CH]
                npair = E[j] // 2
                pending = []
                for pi in range(npair + 1):
                    if pi < npair:
                        sc = sps.tile([P, 1024], F32, tag="sc")
                        at = attn.tile([P, 1024], DT_AT, tag="attnT")
                        for half in range(2):
                            t = 2 * pi + half
                            nc.tensor.matmul(
                                sc[:, half * CH:(half + 1) * CH],
                                kT_sb[:, t * P:(t + 1) * P],
                                qslc,
                                start=True,
                                stop=True,
                            )
                        nc.scalar.activation(
                            out=at, in_=sc, func=Exp, scale=0.125
                        )
                        new_pending = []
                        for half in range(2):
                            t = 2 * pi + half
                            aslc = at[:, half * CH:(half + 1) * CH]
                            if causal and t >= E[j] - 8:
                                m = t - (E[j] - 8)
                                eng = (nc.gpsimd if _CFG["mask_pool"]
                                       else nc.vector)
                                eng.tensor_tensor(
                                    aslc, aslc, masks_sb[:, m, :],
                                    mybir.AluOpType.mult,
                                )
                            new_pending.append((t, aslc))
                    else:
                        new_pending = []
                    for t, aslc in pending:
                        nc.tensor.matmul(
                            oT_ps,
                            v_sb[:, t, :],
                            aslc,
                            start=(t == 0),
                            stop=(t == E[j] - 1),
                            skip_group_check=True,
                        )
                    pending = new_pending

                # finalize job j
                oT = ostage.tile([DK + 1, CH], F32, tag="oT")
                nc.vector.tensor_copy(oT, oT_ps)
                for blk in range(CH // P):
                    po = pps.tile([P, DK + 1], F32, tag="pps")
                    nc.tensor.transpose(
                        po,
                        in_=oT[:, blk * P:(blk + 1) * P],
                        identity=ident[:DK + 1, :DK + 1],
                    )
                    rec = ostage.tile([P, 1], F32, tag="rec")
                    nc.vector.reciprocal(rec, po[:, DK:DK + 1])
                    ot = ostage.tile([P, DK], F32, tag="ot")
                    nc.vector.tensor_scalar_mul(ot, po[:, 0:DK], rec)
                    r0 = j * CH + blk * P
                    nc.sync.dma_start(out=out[r0:r0 + P, :], in_=ot)

            # ---------------- interleaved emission: group j feeds job j
            for j in range(NJ):
                proj_q_chunk(j)
                lo = 2 * j if causal else (j * (S // CH) // NJ)
                hi = 2 * j + 2 if causal else ((j + 1) * (S // CH) // NJ)
                for ch in range(lo, hi):
                    proj_kv_chunk(ch)
                    for blk in range(CH // P):
                        transpose_v(ch * (CH // P) + blk)
                if not causal and j == 0:
                    # non-causal: every job needs all keys; project the rest
                    # before the first job runs
                    for ch in range(hi, S // CH):
                        proj_kv_chunk(ch)
                        for blk in range(CH // P):
                            transpose_v(ch * (CH // P) + blk)
                attention_job(j)

    _split_sync_waits(nc)
    return nc


_PROGRAMS = {}


def _program(causal: bool):
    if causal not in _PROGRAMS:
        _PROGRAMS[causal] = _build_program(causal)
    return _PROGRAMS[causal]


def _host_masks(parity: int) -> np.ndarray:
    """masks[m] multiplies the exp'd [sk=128, sq=512] tile of the job whose
    diagonal band covers key tiles [E-8, E); m = position in that band."""
    sk = np.arange(P)[:, None]
    sq = np.arange(CH)[None, :]
    m = np.zeros((8, P, CH), np.float32)
    for i in range(8):
        if parity == 1:
            if i < 4:
                m[i] = 1.0
            else:
                r = i - 4
                m[i] = (sq >= r * P + sk).astype(np.float32)
        else:
            if i < 4:
                m[i] = (sq >= i * P + sk).astype(np.float32)
            else:
                m[i] = 0.0
    return m


def _chunked(xt_rows: np.ndarray, np_x) -> np.ndarray:
    """[rows, D] -> [nch, 128, KD*CH] where [ch, p, kd*CH+s] =
    x[ch*CH+s, kd*128+p]."""
    nch = xt_rows.shape[0] // CH
    a = xt_rows.reshape(nch, CH, KD, P).transpose(0, 3, 2, 1)
    return np.ascontiguousarray(a.reshape(nch, P, KD * CH).astype(np_x))


def kernel(x1, x2, Wq, bq, Wk, bk, Wv, bv, apply_mask):
    np_x = _NPMAP[_CFG["dt_proj"]]
    np_at = _NPMAP[_CFG["dt_pv"]]
    x1 = np.asarray(x1, dtype=np.float32)
    x2 = np.asarray(x2, dtype=np.float32)
    Wq_h = np.ascontiguousarray(np.asarray(Wq, np.float32).astype(np_x))
    Wk_h = np.ascontiguousarray(np.asarray(Wk, np.float32).astype(np_x))
    Wv_h = np.ascontiguousarray(np.asarray(Wv, np.float32).astype(np_x))
    bq_h = np.ascontiguousarray(np.asarray(bq, np.float32))
    bk_h = np.ascontiguousarray(np.asarray(bk, np.float32))
    bv_h = np.ascontiguousarray(np.asarray(bv, np.float32))
    causal = bool(int(np.asarray(apply_mask)))

    nc = _program(causal)

    x2c_h = [_chunked(x2[b], np_x) for b in range(B)]
    ones_h = np.ones((P, NSK), np_at)
    masks_h = [_host_masks(p).astype(np_at) for p in range(2)]

    in_maps = []
    for core in range(N_CORES):
        b, p = core // 2, core % 2
        xb = x1[b]                                   # [S, D]
        rows = np.concatenate(
            [xb[(2 * j + p) * CH:(2 * j + p + 1) * CH] for j in range(NJ)], axis=0
        )                                            # [2048, D]
        in_maps.append({
            "x1c": _chunked(rows, np_x),
            "x2c": x2c_h[b],
            "wq": Wq_h, "wk": Wk_h, "wv": Wv_h,
            "bq": bq_h, "bk": bk_h, "bv": bv_h,
            "masks": masks_h[p],
            "ones": ones_h,
        })

    res = run_bass_kernel_spmd(
        nc, in_maps, core_ids=list(range(N_CORES)), trace=_CFG["trace"]
    )
    kernel.last_result = res

    outp = np.empty((B, S, DK), np.float32)
    for core in range(N_CORES):
        b, p = core // 2, core % 2
        o = res.results[core]["out"]                 # [2048, 64]
        for j in range(NJ):
            outp[b, (2 * j + p) * CH:(2 * j + p + 1) * CH] = \
                o[j * CH:(j + 1) * CH]
    return outp


# revision 17
# speedup vs baseline: 1.5756x; 1.1706x over previous
"""Causal single-head attention (B=4, S=4096, D=512, dk=64) on 8 Trainium2
NeuronCores via Bass/Tile.

Sharding: core c handles batch b = c//2, query parity p = c%2 — the four
512-row query chunks with global chunk index 2j+p, j=0..3.  Work per job j
is uniform across cores (E[j] = 8j+8 key-tiles of 128); causal boundary
differences between parities are handled by per-core 0/1 mask tensors
(data, not program), so a single SPMD program serves all 8 cores.

Per-core pipeline (emission interleaves projection chunks with attention
jobs so the in-order PE stream stays dense and the HAM clock-gate keeps
the PE at 2.4 GHz):

  group j:  project qT chunk j;  project kT/vT for x2 chunks 2j, 2j+1;
            PE-transpose vT tiles into v_aug [128, 65] tiles (column 64
            is 1.0 so the PV matmul also accumulates the softmax
            denominator);  then run attention job j over key tiles
            t < E[j], two tiles per scores-PSUM buffer:
               scT(t,t+1) = kT_tile^T qT_j          (PE -> PSUM pair)
               attnT = exp(scT / 8)                 (ACT; no max needed,
                                                     scores in [0, ~6])
               diagonal-band tiles multiply by a mask tile (DVE)
               outT[j] += v_aug(t)^T attnT(t)       (PE, deferred one
                                                     pair so PE never
                                                     waits on exp)
            finalize: PE-transpose outT, divide by denominator column
            (DVE reciprocal + per-partition multiply), DMA out.

Matmul operands are bf16 (PE runs 1 cycle/row; fp32 is 4 and float32r is
SBUF-bandwidth-capped on this toolchain); accumulation is fp32 in PSUM.
"""
import os
import numpy as np
import ml_dtypes

import bass_rust
import concourse.bass as bass
import concourse.tile as tile
from concourse import mybir
from concourse.bass_utils import run_bass_kernel_spmd
from concourse.masks import make_identity

# ---------------------------------------------------------------- constants
P = 128          # partitions / sk tile
D = 512          # model dim
DK = 64          # key dim
S = 4096         # sequence
B = 4            # batch
CH = 512         # sq chunk width (one job)
NJ = 4           # jobs per core
KD = D // P      # k-tiles in the D contraction
NSK = S // P     # sk tiles
SQ = NJ * CH     # q rows per core
N_CORES = 8

F32 = mybir.dt.float32
BF16 = mybir.dt.bfloat16
F32R = mybir.dt.float32r

_DTMAP = {"bf16": BF16, "f32r": F32R, "f32": F32}
_NPMAP = {"bf16": ml_dtypes.bfloat16, "f32r": np.float32, "f32": np.float32}

_CFG = {
    "dt_proj": os.environ.get("K_DT_PROJ", "bf16"),
    "dt_sc": os.environ.get("K_DT_SC", "bf16"),
    "dt_pv": os.environ.get("K_DT_PV", "bf16"),
    "relu_dve": os.environ.get("K_RELU_DVE", "1") == "1",
    "mask_pool": os.environ.get("K_MASK_POOL", "0") == "1",
    "trace": os.environ.get("K_TRACE", "0") == "1",
}


# ------------------------------------------------- walrus codegen workarounds
def _patch_tile_drain():
    """This neuronxcc rejects >1 sync wait on a CTRL (Drain) instruction;
    TileContext's tail drain carries one wait per live semaphore.  Split the
    waits onto dedicated SP nops, one wait each."""
    from concourse.tile import TileContext

    if getattr(TileContext, "_drain_patched", False):
        return

    def _patched(self, tick_clock, wait_clock):
        nc = self.nc
        probe = nc.sync.nop(nofuse=True, hint="tail_wait_probe")
        wait_clock.add_sem_waits(
            probe.ins, bass_rust.ScopedClock({None: tick_clock.global_clock})
        )
        si = probe.ins.sync_info
        waits = list(si.on_wait) if si is not None else []
        probe.ins.sync_info = bass_rust.SyncInfo(on_wait=waits[:1], on_update=[])
        for w in waits[1:]:
            carrier = nc.sync.nop(nofuse=True, hint="tail_wait")
            carrier.ins.sync_info = bass_rust.SyncInfo(on_wait=[w], on_update=[])
        nc.sync.drain()

        nc.all_engine_barrier()
        assert self.sems is not None
        popped = nc._tile_sem_poison_stack.pop()
        assert popped is self._sem_poison
        nc.clear_and_free_semaphores(list(self.sems.allocated().values()))
        nc.all_engine_barrier()

    TileContext._drain_and_barrier = _patched
    TileContext._drain_patched = True


def _split_sync_waits(nc, max_waits: int = 1):
    """walrus here rejects >1 sync wait on at least CTRL and S3_LW (weight
    load) instruction structs.  Hoist excess waits onto same-engine NOPs
    placed immediately before the instruction (engine streams execute block
    order, so the waits still gate the instruction)."""
    counter = [0]
    for fn in nc.m.functions:
        for bb in fn.blocks:
            changed = False
            new = []
            for inst in bb.instructions:
                si = inst.sync_info
                waits = list(si.on_wait) if si is not None else []
                if len(waits) > max_waits:
                    changed = True
                    for w in waits[:-max_waits]:
                        counter[0] += 1
                        nop = bass_rust.InstNoOp(
                            name=f"I-waitsplit-{counter[0]}", engine=inst.engine
                        )
                        nop.bass_nofuse = True
                        nop.sync_info = bass_rust.SyncInfo(
                            on_wait=[w], on_update=[]
                        )
                        new.append(nop)
                    inst.sync_info = bass_rust.SyncInfo(
                        on_wait=waits[-max_waits:], on_update=list(si.on_update)
                    )
                new.append(inst)
            if changed:
                bb.instructions = new


# ---------------------------------------------------------------- program
def _build_program(causal: bool):
    _patch_tile_drain()
    nc = bass.Bass()

    DT_X = _DTMAP[_CFG["dt_proj"]]
    DT_QK = _DTMAP[_CFG["dt_sc"]]
    DT_AT = _DTMAP[_CFG["dt_pv"]]

    # chunk-contiguous host layouts: one DMA per 512-column chunk, each a
    # fully contiguous [128, KD*CH] block
    x1c = nc.declare_dram_parameter("x1c", [SQ // CH, P, KD * CH], DT_X,
                                    isOutput=False)
    x2c = nc.declare_dram_parameter("x2c", [S // CH, P, KD * CH], DT_X,
                                    isOutput=False)
    wq = nc.declare_dram_parameter("wq", [D, DK], DT_X, isOutput=False)
    wk = nc.declare_dram_parameter("wk", [D, DK], DT_X, isOutput=False)
    wv = nc.declare_dram_parameter("wv", [D, DK], DT_X, isOutput=False)
    bq = nc.declare_dram_parameter("bq", [DK], F32, isOutput=False)
    bk = nc.declare_dram_parameter("bk", [DK], F32, isOutput=False)
    bv = nc.declare_dram_parameter("bv", [DK], F32, isOutput=False)
    masks = nc.declare_dram_parameter("masks", [8, P, CH], DT_AT, isOutput=False)
    ones = nc.declare_dram_parameter("ones", [P, NSK], DT_AT, isOutput=False)
    out = nc.declare_dram_parameter("out", [SQ, DK], F32, isOutput=True)

    E = [8 * j + 8 for j in range(NJ)] if causal else [NSK] * NJ

    Exp = mybir.ActivationFunctionType.Exp
    Relu = mybir.ActivationFunctionType.Relu

    def bias_relu(dst, src_psum, bias_sb):
        """dst = relu(src + bias), bias per-partition [p,1]."""
        if _CFG["relu_dve"]:
            nc.vector.tensor_scalar(
                dst, src_psum, bias_sb, 0.0,
                mybir.AluOpType.add, mybir.AluOpType.max,
            )
        else:
            nc.scalar.activation(out=dst, in_=src_psum, func=Relu,
                                 bias=bias_sb, scale=1.0)

    with tile.TileContext(nc) as tc:
        with (
            tc.tile_pool(name="const", bufs=1) as const,
            tc.tile_pool(name="xin", bufs=8) as xin,
            tc.tile_pool(name="resident", bufs=1) as res,
            tc.tile_pool(name="attn", bufs=4) as attn,
            tc.tile_pool(name="ostage", bufs=4) as ostage,
            tc.tile_pool(name="outps", bufs=2, space="PSUM") as outps,
            tc.tile_pool(name="pps", bufs=2, space="PSUM") as pps,
            tc.tile_pool(name="sps", bufs=2, space="PSUM") as sps,
        ):
            # ---------------- constants
            wq_sb = const.tile([P, KD, DK], DT_X)
            wk_sb = const.tile([P, KD, DK], DT_X)
            wv_sb = const.tile([P, KD, DK], DT_X)
            nc.sync.dma_start(out=wq_sb, in_=wq.rearrange("(kd p) m -> p kd m", p=P))
            nc.sync.dma_start(out=wk_sb, in_=wk.rearrange("(kd p) m -> p kd m", p=P))
            nc.sync.dma_start(out=wv_sb, in_=wv.rearrange("(kd p) m -> p kd m", p=P))
            bq_sb = const.tile([DK, 1], F32)
            bk_sb = const.tile([DK, 1], F32)
            bv_sb = const.tile([DK, 1], F32)
            nc.sync.dma_start(out=bq_sb, in_=bq.rearrange("(p o) -> p o", o=1))
            nc.sync.dma_start(out=bk_sb, in_=bk.rearrange("(p o) -> p o", o=1))
            nc.sync.dma_start(out=bv_sb, in_=bv.rearrange("(p o) -> p o", o=1))
            identv = const.tile([P, P], DT_X)
            make_identity(nc, identv)
            ident = const.tile([P, P], F32)
            make_identity(nc, ident)

            qT_sb = res.tile([DK, SQ], DT_QK)
            kT_sb = res.tile([DK, S], DT_QK)
            vT_sb = res.tile([DK, S], DT_X)
            v_sb = res.tile([P, NSK, DK + 1], DT_AT)
            nc.sync.dma_start(
                out=v_sb[:, :, DK:DK + 1],
                in_=ones.rearrange("p (n o) -> p n o", o=1),
            )

            # ---- all input DMAs issued up-front (SP executes triggers in
            # program order; interleaving them with output DMAs would stall
            # the input stream behind compute-dependent stores)
            x1_tiles, x2_tiles = [], []
            for ch in range(SQ // CH):
                xt = xin.tile([P, KD, CH], DT_X, name=f"x1t{ch}", tag="x1c")
                nc.sync.dma_start(
                    out=xt, in_=x1c[ch].rearrange("p (kd s) -> p kd s", kd=KD)
                )
                x1_tiles.append(xt)
                for ch2 in (2 * ch, 2 * ch + 1):
                    xt2 = xin.tile([P, KD, CH], DT_X, name=f"x2t{ch2}", tag="x2c")
                    nc.sync.dma_start(
                        out=xt2,
                        in_=x2c[ch2].rearrange("p (kd s) -> p kd s", kd=KD),
                    )
                    x2_tiles.append(xt2)
                if ch == 0 and causal:
                    masks_sb = const.tile([P, 8, CH], DT_AT)
                    nc.sync.dma_start(
                        out=masks_sb, in_=masks.rearrange("m p s -> p m s")
                    )

            def proj_q_chunk(ch):
                xt = x1_tiles[ch]
                pq = pps.tile([DK, CH], F32, tag="pps")
                for kd in range(KD):
                    nc.tensor.matmul(
                        pq, wq_sb[:, kd, :], xt[:, kd, :],
                        start=(kd == 0), stop=(kd == KD - 1),
                    )
                bias_relu(qT_sb[:, ch * CH:(ch + 1) * CH], pq, bq_sb)

            def proj_kv_chunk(ch):
                xt = x2_tiles[ch]
                pk = pps.tile([DK, CH], F32, tag="pps")
                for kd in range(KD):
                    nc.tensor.matmul(
                        pk, wk_sb[:, kd, :], xt[:, kd, :],
                        start=(kd == 0), stop=(kd == KD - 1),
                    )
                bias_relu(kT_sb[:, ch * CH:(ch + 1) * CH], pk, bk_sb)
                pv = pps.tile([DK, CH], F32, tag="pps")
                for kd in range(KD):
                    nc.tensor.matmul(
                        pv, wv_sb[:, kd, :], xt[:, kd, :],
                        start=(kd == 0), stop=(kd == KD - 1),
                    )
                bias_relu(vT_sb[:, ch * CH:(ch + 1) * CH], pv, bv_sb)

            def transpose_v(st):
                pt = pps.tile([P, DK], DT_X, tag="pps")
                nc.tensor.transpose(
                    pt, in_=vT_sb[:, st * P:(st + 1) * P],
                    identity=identv[:DK, :DK],
                )
                nc.vector.tensor_copy(v_sb[:, st, 0:DK], pt)

            def attention_job(j, new_tiles=()):
                oT_ps = outps.tile([DK + 1, CH], F32, tag="outT")
                qslc = qT_sb[:, j * CH:(j + 1) * CH]
                npair = E[j] // 2
                pending = []
                for pi in range(npair + 1):
                    # spread the v transposes of this group's new key tiles
                    # across the early pairs (each tile is ready well before
                    # its PV consumes it)
                    for st in new_tiles[2 * pi:2 * pi + 2]:
                        transpose_v(st)
                    if pi < npair:
                        sc = sps.tile([P, 1024], F32, tag="sc")
                        at = attn.tile([P, 1024], DT_AT, tag="attnT")
                        for half in range(2):
                            t = 2 * pi + half
                            nc.tensor.matmul(
                                sc[:, half * CH:(half + 1) * CH],
                                kT_sb[:, t * P:(t + 1) * P],
                                qslc,
                                start=True,
                                stop=True,
                            )
                        nc.scalar.activation(
                            out=at, in_=sc, func=Exp, scale=0.125
                        )
                        new_pending = []
                        for half in range(2):
                            t = 2 * pi + half
                            aslc = at[:, half * CH:(half + 1) * CH]
                            if causal and t >= E[j] - 8:
                                m = t - (E[j] - 8)
                                eng = (nc.gpsimd if _CFG["mask_pool"]
                                       else nc.vector)
                                eng.tensor_tensor(
                                    aslc, aslc, masks_sb[:, m, :],
                                    mybir.AluOpType.mult,
                                )
                            new_pending.append((t, aslc))
                    else:
                        new_pending = []
                    for t, aslc in pending:
                        nc.tensor.matmul(
                            oT_ps,
                            v_sb[:, t, :],
                            aslc,
                            start=(t == 0),
                            stop=(t == E[j] - 1),
                            skip_group_check=True,
                        )
                    pending = new_pending

                # finalize job j
                oT = ostage.tile([DK + 1, CH], F32, tag="oT")
                nc.vector.tensor_copy(oT, oT_ps)
                for blk in range(CH // P):
                    po = pps.tile([P, DK + 1], F32, tag="pps")
                    nc.tensor.transpose(
                        po,
                        in_=oT[:, blk * P:(blk + 1) * P],
                        identity=ident[:DK + 1, :DK + 1],
                    )
                    rec = ostage.tile([P, 1], F32, tag="rec")
                    nc.vector.reciprocal(rec, po[:, DK:DK + 1])
                    ot = ostage.tile([P, DK], F32, tag="ot")
                    nc.vector.tensor_scalar_mul(ot, po[:, 0:DK], rec)
                    r0 = j * CH + blk * P
                    nc.sync.dma_start(out=out[r0:r0 + P, :], in_=ot)

            # ---------------- interleaved emission: group j feeds job j
            for j in range(NJ):
                proj_q_chunk(j)
                lo, hi = 2 * j, 2 * j + 2
                if not causal:
                    lo, hi = (0, S // CH) if j == 0 else (0, 0)
                new_tiles = []
                for ch in range(lo, hi):
                    proj_kv_chunk(ch)
                    new_tiles.extend(
                        ch * (CH // P) + blk for blk in range(CH // P)
                    )
                if not causal and j == 0:
                    # all keys needed up-front: transpose before the job
                    for st in new_tiles:
                        transpose_v(st)
                    new_tiles = []
                attention_job(j, new_tiles)

    _split_sync_waits(nc)
    return nc


_PROGRAMS = {}


def _program(causal: bool):
    if causal not in _PROGRAMS:
        _PROGRAMS[causal] = _build_program(causal)
    return _PROGRAMS[causal]


def _host_masks(parity: int) -> np.ndarray:
    """masks[m] multiplies the exp'd [sk=128, sq=512] tile of the job whose
    diagonal band covers key tiles [E-8, E); m = position in that band."""
    sk = np.arange(P)[:, None]
    sq = np.arange(CH)[None, :]
    m = np.zeros((8, P, CH), np.float32)
    for i in range(8):
        if parity == 1:
            if i < 4:
                m[i] = 1.0
            else:
                r = i - 4
                m[i] = (sq >= r * P + sk).astype(np.float32)
        else:
            if i < 4:
                m[i] = (sq >= i * P + sk).astype(np.float32)
            else:
                m[i] = 0.0
    return m


def _chunked(xt_rows: np.ndarray, np_x) -> np.ndarray:
    """[rows, D] -> [nch, 128, KD*CH] where [ch, p, kd*CH+s] =
    x[ch*CH+s, kd*128+p]."""
    nch = xt_rows.shape[0] // CH
    a = xt_rows.reshape(nch, CH, KD, P).transpose(0, 3, 2, 1)
    return np.ascontiguousarray(a.reshape(nch, P, KD * CH).astype(np_x))


def kernel(x1, x2, Wq, bq, Wk, bk, Wv, bv, apply_mask):
    np_x = _NPMAP[_CFG["dt_proj"]]
    np_at = _NPMAP[_CFG["dt_pv"]]
    x1 = np.asarray(x1, dtype=np.float32)
    x2 = np.asarray(x2, dtype=np.float32)
    Wq_h = np.ascontiguousarray(np.asarray(Wq, np.float32).astype(np_x))
    Wk_h = np.ascontiguousarray(np.asarray(Wk, np.float32).astype(np_x))
    Wv_h = np.ascontiguousarray(np.asarray(Wv, np.float32).astype(np_x))
    bq_h = np.ascontiguousarray(np.asarray(bq, np.float32))
    bk_h = np.ascontiguousarray(np.asarray(bk, np.float32))
    bv_h = np.ascontiguousarray(np.asarray(bv, np.float32))
    causal = bool(int(np.asarray(apply_mask)))

    nc = _program(causal)

    x2c_h = [_chunked(x2[b], np_x) for b in range(B)]
    ones_h = np.ones((P, NSK), np_at)
    masks_h = [_host_masks(p).astype(np_at) for p in range(2)]

    in_maps = []
    for core in range(N_CORES):
        b, p = core // 2, core % 2
        xb = x1[b]                                   # [S, D]
        rows = np.concatenate(
            [xb[(2 * j + p) * CH:(2 * j + p + 1) * CH] for j in range(NJ)], axis=0
        )                                            # [2048, D]
        in_maps.append({
            "x1c": _chunked(rows, np_x),
            "x2c": x2c_h[b],
            "wq": Wq_h, "wk": Wk_h, "wv": Wv_h,
            "bq": bq_h, "bk": bk_h, "bv": bv_h,
            "masks": masks_h[p],
            "ones": ones_h,
        })

    res = run_bass_kernel_spmd(
        nc, in_maps, core_ids=list(range(N_CORES)), trace=_CFG["trace"]
    )
    kernel.last_result = res

    outp = np.empty((B, S, DK), np.float32)
    for core in range(N_CORES):
        b, p = core // 2, core % 2
        o = res.results[core]["out"]                 # [2048, 64]
        for j in range(NJ):
            outp[b, (2 * j + p) * CH:(2 * j + p + 1) * CH] = \
                o[j * CH:(j + 1) * CH]
    return outp
